# revision 17
# baseline (speedup 1.0000x reference)
"""DAG-GNN recommender forward pass on 8 Trainium2 NeuronCores (Bass/Tile).

Strategy (per core, nodes sharded 8 ways):
  - Reorder h[src] @ W -> (A @ h) @ W: aggregate raw h rows first, then one
    dense [H,H] matmul per message type (turns 42 GFLOP of per-edge matmuls
    into 2.6 GFLOP per layer).
  - Edges sorted by output node on host (index preprocessing only); the
    per-128-node-block segment sum runs on the PE as a chain of one-hot
    matmuls accumulating in PSUM (fp8 one-hot stationary operand with exact
    0/1 entries, bf16 gathered rows as the moving operand).
  - Gathered h rows fetched from a replicated full bf16 copy of h in HBM via
    the custom SWDGE dma_gather (512 B rows).
  - Degree normalization = per-partition scale on the PSUM->SBUF copy.
  - After each layer the new h shard is AllGathered (bf16) for the next
    layer's gathers.
All floating-point arithmetic runs on device; the host only relabels/sorts
integer indices, builds the 0/1 one-hot tables, computes 1/deg from integer
counts, replicates bias rows, and casts dtypes.
"""

import numpy as np
import ml_dtypes

P = 128
N, E, IN, H, OUT, L = 20000, 320000, 256, 256, 128, 3
NC = 8
SHARD = N // NC            # 2500
NBLK = (SHARD + P - 1) // P  # 20 (last block 68 rows)
EPS = 1e-5

_BF16 = ml_dtypes.bfloat16
_FP8 = ml_dtypes.float8_e4m3


def _block_rows(b):
    return min(P, SHARD - b * P)


def preprocess(edge_index):
    """Index-only preprocessing: within-shard node permutation balancing
    per-block edge counts, per-block (out,src) dedup with count-valued
    one-hot entries, padded chunk tables, 1/deg columns."""
    src = np.asarray(edge_index[0]).astype(np.int64)
    dst = np.asarray(edge_index[1]).astype(np.int64)
    deg_p = np.bincount(dst, minlength=N)          # in-degree (per dst)
    deg_c = np.bincount(src, minlength=N)          # out-degree (per src)
    inv_p = (1.0 / np.maximum(deg_p, 1)).astype(np.float32)
    inv_c = (1.0 / np.maximum(deg_c, 1)).astype(np.float32)

    # --- balance blocks: permute nodes within each shard so every block of
    # 128 gets ~equal parent AND child degree (greedy, heaviest first) ---
    newid = np.empty(N, np.int64)   # old global id -> new global id
    load = deg_p + deg_c
    for c in range(NC):
        lo = c * SHARD
        nodes = np.argsort(-load[lo:lo + SHARD], kind="stable") + lo
        cap = [P] * NBLK
        cap[NBLK - 1] = SHARD - (NBLK - 1) * P
        sp = [0] * NBLK
        sc = [0] * NBLK
        fill = [0] * NBLK
        for nd in nodes:
            wp = int(deg_p[nd])
            wc = int(deg_c[nd])
            best, bkey = -1, None
            for b in range(NBLK):
                if fill[b] >= cap[b]:
                    continue
                k = (max(sp[b] + wp, sc[b] + wc), sp[b] + sc[b])
                if bkey is None or k < bkey:
                    best, bkey = b, k
            sp[best] += wp
            sc[best] += wc
            newid[nd] = lo + best * P + fill[best]
            fill[best] += 1
    src_n = newid[src]
    dst_n = newid[dst]

    # per (core, dir, block): deduped (out_local, gather idx, count)
    per = [[None, None] for _ in range(NC)]
    for d, (out_idx, gat_idx) in enumerate(((dst_n, src_n), (src_n, dst_n))):
        key = out_idx * N + gat_idx
        uniq, cnt = np.unique(key, return_counts=True)
        o_u = uniq // N
        g_u = uniq % N
        core_of = o_u // SHARD
        for c in range(NC):
            sel = core_of == c
            per[c][d] = (o_u[sel] - c * SHARD, g_u[sel], cnt[sel])

    # one gather row per DISTINCT src within a (dir, block); one-hot rows are
    # multi-hot (one column per out node the src feeds, value = edge count)
    nchb = 1
    for c in range(NC):
        for d in range(2):
            o, g, _ = per[c][d]
            u2 = np.unique((o // P) * N + g)
            cm = np.bincount(u2 // N, minlength=NBLK)
            nchb = max(nchb, int(np.ceil(cm.max() / P)))

    tot_edges = 2 * NBLK * nchb * P  # padded gather rows per core
    cores = []
    for c in range(NC):
        gidx = np.zeros(tot_edges, np.int16)
        oh = np.zeros((tot_edges, P), _FP8)
        invdeg = np.ones((P, 2 * NBLK), np.float32)
        for d in range(2):
            o, g, w = per[c][d]
            blk = o // P
            starts = np.searchsorted(blk, np.arange(NBLK))
            ends = np.searchsorted(blk, np.arange(NBLK), side="right")
            for b in range(NBLK):
                s, e = int(starts[b]), int(ends[b])
                base = (d * NBLK + b) * nchb * P
                gb = g[s:e]
                ob = o[s:e] - b * P
                wb = w[s:e]
                order = np.argsort(gb, kind="stable")
                gb, ob, wb = gb[order], ob[order], wb[order]
                if len(gb) == 0:
                    continue
                newg = np.empty(len(gb), bool)
                newg[0] = True
                newg[1:] = gb[1:] != gb[:-1]
                rows = np.cumsum(newg) - 1
                gidx[base:base + int(rows[-1]) + 1] = gb[newg]
                oh[base + rows, ob] = wb
            inv = inv_p if d == 0 else inv_c
            col = np.ones(NBLK * P, np.float32)
            ids = np.arange(c * SHARD, (c + 1) * SHARD)
            old_of_new = np.empty(SHARD, np.int64)
            old_of_new[newid[ids] - c * SHARD] = ids
            col[:SHARD] = inv[old_of_new]
            invdeg[:, d * NBLK:(d + 1) * NBLK] = col.reshape(NBLK, P).T
        # wrap-16 idx packing, replicated to 128 partitions
        idx16 = gidx.reshape(-1, 16).T           # [16, tot/16]
        idx_pack = np.tile(idx16, (8, 1)).copy() # [128, tot/16]
        # partition-major one-hot: row p col ch*P+m = oh[ch*P+p, m]
        totch = tot_edges // P
        oh_pm = np.ascontiguousarray(
            oh.reshape(totch, P, P).transpose(1, 0, 2).reshape(P, totch * P))
        ids = np.arange(c * SHARD, (c + 1) * SHARD)
        old_rows = np.empty(SHARD, np.int64)
        old_rows[newid[ids] - c * SHARD] = ids   # new local pos -> old node id
        cores.append({"gidx": idx_pack, "oh": oh_pm, "invdeg": invdeg,
                      "old_rows": old_rows})
    return cores, nchb


def _rep(v, rows=P):
    v = np.asarray(v, np.float32).reshape(1, -1)
    return np.tile(v, (rows, 1)).astype(_BF16)


def build_nc(nchb, mode="full", reps=1,
             stages=("agg", "gather", "ohload", "aggmm", "dense", "ln"),
             gfp8=True):
    """Build the Bass program. mode='full': real collectives, reps must be 1.
    mode='timing': AllGather replaced by 8 local slice copies, whole compute
    wrapped in a For_i(reps) loop. gfp8: replicate h (message path only) in
    fp8e4m3 — halves gather bytes, double-pumps the aggregation matmuls."""
    import concourse.bacc as bacc
    import concourse.mybir as mybir
    import concourse.tile as tile
    from concourse.masks import make_identity

    dt = mybir.dt
    nc = bacc.Bacc("TRN2", target_bir_lowering=False, debug=False, num_devices=NC,
                   num_swdge_queues=4)

    TOTCH = 2 * NBLK * nchb
    # ---- dram I/O ----
    xs_d = nc.dram_tensor("xs", [SHARD, IN], dt.bfloat16, kind="ExternalInput")
    gidx_d = nc.dram_tensor("gidx", [128, TOTCH * 8], dt.int16, kind="ExternalInput")
    oh_d = nc.dram_tensor("oh", [P, TOTCH * P], dt.float8e4, kind="ExternalInput")
    invdeg_d = nc.dram_tensor("invdeg", [P, 2 * NBLK], dt.float32, kind="ExternalInput")
    w_names = (["w_in"] + [f"w_s{l}" for l in range(L)] + [f"w_p{l}" for l in range(L)]
               + [f"w_c{l}" for l in range(L)] + ["w_h1"])
    w_d = {n: nc.dram_tensor(n, [H, H], dt.bfloat16, kind="ExternalInput")
           for n in w_names}
    w_d["w_h2"] = nc.dram_tensor("w_h2", [H, OUT], dt.bfloat16, kind="ExternalInput")
    rep_names = (["in_b"] + [f"self_b{l}" for l in range(L)]
                 + [f"ln_g{l}" for l in range(L)] + [f"ln_b{l}" for l in range(L)]
                 + ["h_b1"])
    rep_d = {n: nc.dram_tensor(n, [P, H], dt.bfloat16, kind="ExternalInput")
             for n in rep_names}
    rep_d["h_b2"] = nc.dram_tensor("h_b2", [P, OUT], dt.bfloat16, kind="ExternalInput")
    y_d = nc.dram_tensor("y", [SHARD, OUT], dt.float32, kind="ExternalOutput")
    gdt = dt.float8e4 if gfp8 else dt.bfloat16
    bounce = nc.dram_tensor("bounce", [SHARD, H], gdt)
    h_full = nc.dram_tensor("h_full", [N, H], gdt, addr_space="Shared")

    with tile.TileContext(nc) as tc:
        with tc.tile_pool(name="const", bufs=1) as cpool, \
             tc.tile_pool(name="gather", bufs=3) as gpool, \
             tc.tile_pool(name="work", bufs=3) as wpool, \
             tc.tile_pool(name="small", bufs=4) as spool, \
             tc.tile_pool(name="psA", bufs=2, space="PSUM") as psA, \
             tc.tile_pool(name="psT", bufs=2, space="PSUM") as psT, \
             tc.tile_pool(name="psD", bufs=2, space="PSUM") as psD:

            # ---- persistent constants ----
            idx_sb = cpool.tile([128, TOTCH * 8], dt.int16)
            nc.sync.dma_start(out=idx_sb[:], in_=gidx_d[:])
            # one-hot tables are layer- and iteration-invariant: persist in
            # SBUF (~80KB/partition) instead of re-streaming 10.5MB per layer
            oh_all = None
            if "ohload" in stages:
                oh_all = cpool.tile([P, TOTCH * P], dt.float8e4, tag="oh_all")
                nc.sync.dma_start(out=oh_all[:], in_=oh_d[:])
            invdeg_sb = cpool.tile([P, 2 * NBLK], dt.float32)
            nc.sync.dma_start(out=invdeg_sb[:], in_=invdeg_d[:])
            ident = cpool.tile([P, P], dt.bfloat16)
            make_identity(nc, ident[:])
            w_sb = {}
            for n, d_ in w_d.items():
                cols = d_.shape[1]
                t = cpool.tile([P, 2, cols], dt.bfloat16, tag=f"w_{n}")
                nc.sync.dma_start(out=t[:], in_=d_[:].rearrange("(k p) n -> p k n", p=P))
                w_sb[n] = t
            rep_sb = {}
            for n, d_ in rep_d.items():
                t = cpool.tile([P, d_.shape[1]], dt.bfloat16, tag=f"r_{n}")
                nc.sync.dma_start(out=t[:], in_=d_[:])
                rep_sb[n] = t
            h_shard = cpool.tile([P, NBLK, H], dt.bfloat16)
            # ablation stand-ins: persistent zeroed tiles so removed stages
            # don't leave consumers reading unwritten pool tiles
            gfake = ohfake = mfake = None
            if "gather" not in stages:
                gfake = cpool.tile([P, nchb, H], gdt, tag="gfake")
                nc.vector.memset(gfake[:], 0)
            if "ohload" not in stages:
                ohfake = cpool.tile([P, nchb * P], dt.float8e4, tag="ohfake")
                nc.vector.memset(ohfake[:], 0)
            if "aggmm" not in stages:
                mfake = cpool.tile([P, H], dt.bfloat16, tag="mfake")
                nc.vector.memset(mfake[:], 0)

            def allgather():
                if mode == "full":
                    nc.gpsimd.collective_compute(
                        "AllGather", mybir.AluOpType.bypass,
                        replica_groups=[list(range(NC))],
                        ins=[bounce.ap().opt()], outs=[h_full.ap().opt()],
                    )
                else:
                    for c in range(NC):
                        nc.sync.dma_start(
                            out=h_full[c * SHARD:(c + 1) * SHARD, :], in_=bounce[:])

            def transpose_to(dst_sb, src_ap):
                """dst_sb[:, k*128:(k+1)*128] = src 128x128 halves transposed."""
                nhalf = src_ap.shape[-1] // P
                for k in range(nhalf):
                    tp = psT.tile([P, P], dt.bfloat16, space="PSUM", tag="tp")
                    nc.tensor.transpose(tp[:], src_ap[:, k * P:(k + 1) * P], ident[:])
                    nc.vector.tensor_copy(dst_sb[:, k * P:(k + 1) * P], tp[:])

            qrr = [0]  # round-robin swdge queue assignment for gathers

            def aggregate(d, b):
                """one direction's message aggregation for block b -> bf16 tile."""
                base_ch = (d * NBLK + b) * nchb
                g = gfake if gfake is not None else gpool.tile(
                    [P, nchb, H], gdt, tag=f"g{d}")
                if "gather" in stages:
                    # <=1024-idx single-packet calls spread round-robin over
                    # all 4 SWDGE queues (measured ~2x: one queue's ring
                    # serializes descriptor processing)
                    gsp = 8
                    for c0 in range(0, nchb, gsp):
                        c1 = min(c0 + gsp, nchb)
                        nc.gpsimd.dma_gather(
                            g[:, c0:c1, :], h_full[:],
                            idx_sb[:, (base_ch + c0) * 8:(base_ch + c1) * 8],
                            (c1 - c0) * P, (c1 - c0) * P, H,
                            single_packet=(c1 - c0) <= 8,
                            queue_num=qrr[0] % 4)
                        qrr[0] += 1
                if "ohload" in stages:
                    ohs = oh_all[:, base_ch * P:(base_ch + nchb) * P]
                else:
                    ohs = ohfake[:]
                msg = mfake if mfake is not None else wpool.tile(
                    [P, H], dt.bfloat16, tag=f"msg{d}")
                if "aggmm" in stages:
                    ps = psA.tile([P, H], dt.float32, space="PSUM", tag=f"agg{d}")
                    for ch in range(nchb):
                        nc.tensor.matmul(out=ps[:], lhsT=ohs[:, ch * P:(ch + 1) * P],
                                         rhs=g[:, ch, :],
                                         start=(ch == 0), stop=(ch == nchb - 1))
                    col = d * NBLK + b
                    nc.vector.tensor_scalar_mul(msg[:], ps[:],
                                                invdeg_sb[:, col:col + 1])
                return msg

            def dense_block(ps, pairs, start=True):
                """accumulate sum_k lhsT_k.T @ rhs_k into ps."""
                n = len(pairs)
                for i, (lhsT, rhs) in enumerate(pairs):
                    nc.tensor.matmul(out=ps[:], lhsT=lhsT, rhs=rhs,
                                     start=(start and i == 0), stop=(i == n - 1))

            def layer_norm(res, rsum, g_rep, b_rep, out_sb):
                ssq = spool.tile([P, 1], dt.float32, tag="ssq")
                sq = wpool.tile([P, H], dt.bfloat16, tag="sq")
                nc.scalar.activation(sq[:], res[:], mybir.ActivationFunctionType.Square,
                                     accum_out=ssq[:, :1])
                mu = spool.tile([P, 1], dt.float32, tag="mu")
                nc.vector.tensor_scalar_mul(mu[:], rsum[:], 1.0 / H)
                var = spool.tile([P, 1], dt.float32, tag="var")
                nc.vector.tensor_scalar_mul(var[:], ssq[:], 1.0 / H)
                musq = spool.tile([P, 1], dt.float32, tag="musq")
                nc.vector.tensor_mul(musq[:], mu[:], mu[:])
                nc.vector.tensor_sub(var[:], var[:], musq[:])
                nc.vector.tensor_scalar_add(var[:], var[:], EPS)
                std = spool.tile([P, 1], dt.float32, tag="std")
                nc.scalar.activation(std[:], var[:], mybir.ActivationFunctionType.Sqrt)
                rstd = spool.tile([P, 1], dt.float32, tag="rstd")
                nc.vector.reciprocal(rstd[:], std[:])
                nbias = spool.tile([P, 1], dt.float32, tag="nbias")
                nc.vector.tensor_mul(nbias[:], mu[:], rstd[:])
                nc.vector.tensor_scalar_mul(nbias[:], nbias[:], -1.0)
                norm = wpool.tile([P, H], dt.bfloat16, tag="norm")
                nc.scalar.activation(norm[:], res[:],
                                     mybir.ActivationFunctionType.Identity,
                                     bias=nbias[:, :1], scale=rstd[:, :1])
                nc.vector.tensor_mul(norm[:], norm[:], g_rep[:])
                nc.vector.tensor_add(out_sb, norm[:], b_rep[:])

            def body():
                # ---- input projection ----
                for b in range(NBLK):
                    rows = _block_rows(b)
                    xt = wpool.tile([P, IN], dt.bfloat16, tag="xt")
                    if rows < P:
                        nc.vector.memset(xt[:], 0)
                    nc.sync.dma_start(out=xt[:rows, :], in_=xs_d[b * P:b * P + rows, :])
                    xT = wpool.tile([P, 2 * P], dt.bfloat16, tag="xT")
                    transpose_to(xT, xt[:])
                    ps = psD.tile([P, H], dt.float32, space="PSUM", tag="dense")
                    dense_block(ps, [(xT[:, k * P:(k + 1) * P], w_sb["w_in"][:, k, :])
                                     for k in range(2)])
                    nc.vector.tensor_add(h_shard[:, b, :], ps[:], rep_sb["in_b"][:])
                    h8 = wpool.tile([P, H], gdt, tag="h8")
                    nc.vector.tensor_copy(h8[:], h_shard[:, b, :])
                    nc.sync.dma_start(out=bounce[b * P:b * P + rows, :],
                                      in_=h8[:rows, :])
                allgather()

                # ---- message passing layers ----
                for l in range(L):
                    for b in range(NBLK):
                        rows = _block_rows(b)
                        if "agg" in stages:
                            msgP = aggregate(0, b)
                            msgC = aggregate(1, b)
                        if "dense" in stages:
                            hT = wpool.tile([P, 2 * P], dt.bfloat16, tag="hT")
                            transpose_to(hT, h_shard[:, b, :])
                            ps = psD.tile([P, H], dt.float32, space="PSUM", tag="dense")
                            pairs = [(hT[:, k * P:(k + 1) * P], w_sb[f"w_s{l}"][:, k, :])
                                     for k in range(2)]
                            if "agg" in stages:
                                pT = wpool.tile([P, 2 * P], dt.bfloat16, tag="pT")
                                transpose_to(pT, msgP[:])
                                cT = wpool.tile([P, 2 * P], dt.bfloat16, tag="cT")
                                transpose_to(cT, msgC[:])
                                pairs += [(pT[:, k * P:(k + 1) * P],
                                           w_sb[f"w_p{l}"][:, k, :]) for k in range(2)]
                                pairs += [(cT[:, k * P:(k + 1) * P],
                                           w_sb[f"w_c{l}"][:, k, :]) for k in range(2)]
                            dense_block(ps, pairs)
                            pre = wpool.tile([P, H], dt.bfloat16, tag="pre")
                            nc.vector.tensor_add(pre[:], ps[:], rep_sb[f"self_b{l}"][:])
                            act = wpool.tile([P, H], dt.bfloat16, tag="act")
                            nc.scalar.activation(act[:], pre[:],
                                                 mybir.ActivationFunctionType.Gelu)
                            res = wpool.tile([P, H], dt.bfloat16, tag="res")
                            rsum = spool.tile([P, 1], dt.float32, tag="rsum")
                            if "ttr" in stages:
                                nc.vector.tensor_tensor_reduce(
                                    res[:], act[:], h_shard[:, b, :], 1.0, 0.0,
                                    mybir.AluOpType.add, mybir.AluOpType.add,
                                    accum_out=rsum[:, :1])
                            else:
                                nc.vector.tensor_add(res[:], act[:], h_shard[:, b, :])
                                tmp = wpool.tile([P, H], dt.bfloat16, tag="rtmp")
                                nc.vector.tensor_scalar(
                                    tmp[:], res[:], 0.0, None, mybir.AluOpType.add,
                                    op1=mybir.AluOpType.add, accum_out=rsum[:, :1])
                            if "ln" in stages:
                                layer_norm(res, rsum, rep_sb[f"ln_g{l}"],
                                           rep_sb[f"ln_b{l}"], h_shard[:, b, :])
                            else:
                                nc.vector.tensor_copy(h_shard[:, b, :], res[:])
                            if l < L - 1:
                                h8 = wpool.tile([P, H], gdt, tag="h8")
                                nc.vector.tensor_copy(h8[:], h_shard[:, b, :])
                                nc.sync.dma_start(out=bounce[b * P:b * P + rows, :],
                                                  in_=h8[:rows, :])
                    if l < L - 1 and "dense" in stages:
                        allgather()

                # ---- head ----
                for b in range(NBLK):
                    rows = _block_rows(b)
                    hT = wpool.tile([P, 2 * P], dt.bfloat16, tag="hT")
                    transpose_to(hT, h_shard[:, b, :])
                    ps1 = psD.tile([P, H], dt.float32, space="PSUM", tag="dense")
                    dense_block(ps1, [(hT[:, k * P:(k + 1) * P], w_sb["w_h1"][:, k, :])
                                      for k in range(2)])
                    y1 = wpool.tile([P, H], dt.bfloat16, tag="y1")
                    nc.vector.tensor_add(y1[:], ps1[:], rep_sb["h_b1"][:])
                    y1a = wpool.tile([P, H], dt.bfloat16, tag="y1a")
                    nc.scalar.activation(y1a[:], y1[:],
                                         mybir.ActivationFunctionType.Gelu)
                    yT = wpool.tile([P, 2 * P], dt.bfloat16, tag="yT")
                    transpose_to(yT, y1a[:])
                    ps2 = psD.tile([P, OUT], dt.float32, space="PSUM", tag="dense")
                    dense_block(ps2, [(yT[:, k * P:(k + 1) * P], w_sb["w_h2"][:, k, :])
                                      for k in range(2)])
                    yo = wpool.tile([P, OUT], dt.float32, tag="yo")
                    nc.vector.tensor_add(yo[:], ps2[:], rep_sb["h_b2"][:])
                    nc.sync.dma_start(out=y_d[b * P:b * P + rows, :], in_=yo[:rows, :])

            if mode == "timing" and reps > 1:
                with tc.For_i(0, reps, 1):
                    body()
            else:
                body()

    nc.compile()
    return nc


def make_in_maps(inputs, cores):
    x = np.asarray(inputs["x"])
    wm = {
        "w_in": inputs["in_w"], "w_h1": inputs["head_w1"], "w_h2": inputs["head_w2"],
    }
    repm = {"in_b": inputs["in_b"], "h_b1": inputs["head_b1"], "h_b2": inputs["head_b2"]}
    for l in range(L):
        wm[f"w_s{l}"] = inputs["self_w"][l]
        wm[f"w_p{l}"] = inputs["parent_w"][l]
        wm[f"w_c{l}"] = inputs["child_w"][l]
        repm[f"self_b{l}"] = inputs["self_b"][l]
        repm[f"ln_g{l}"] = inputs["ln_g"][l]
        repm[f"ln_b{l}"] = inputs["ln_b"][l]
    w_np = {k: np.asarray(v, np.float32).astype(_BF16) for k, v in wm.items()}
    rep_np = {k: _rep(v) for k, v in repm.items()}
    in_maps = []
    for c in range(NC):
        m = {"xs": np.ascontiguousarray(x[cores[c]["old_rows"]]).astype(_BF16),
             "gidx": cores[c]["gidx"], "oh": cores[c]["oh"],
             "invdeg": cores[c]["invdeg"]}
        m.update(w_np)
        m.update(rep_np)
        in_maps.append(m)
    return in_maps


_CACHE = {}


def kernel(**inputs):
    from concourse.bass_utils import run_bass_kernel_spmd

    cores, nchb = preprocess(inputs["edge_index"])
    key = ("full", nchb)
    if key not in _CACHE:
        _CACHE[key] = build_nc(nchb, mode="full")
    nc = _CACHE[key]
    in_maps = make_in_maps(inputs, cores)
    res = run_bass_kernel_spmd(nc, in_maps, core_ids=list(range(NC)))
    y_cat = np.concatenate([res.results[c]["y"] for c in range(NC)], axis=0)
    old_of_new = np.concatenate([cores[c]["old_rows"] for c in range(NC)])
    y = np.empty_like(y_cat)
    y[old_of_new] = y_cat
    return y

